# revision 23
# baseline (speedup 1.0000x reference)
"""Causal self-attention (B=4, N=2048, D=1024, H=16, hd=64) on 8 trn2 cores.

Sharding: core c -> (batch b = c//2, head-group hg = c%2 of 8 heads).
Each core computes, for its batch and its 8 heads (processed as 4 pairs):
  projT = (x[b] @ W_pair + bias)^T via bf16 matmuls (lhsT = W tiles,
    rhs = xT); rows are [K_h1|K_h2], [Q_h1|Q_h2], [V_h1|V_h2] (Q 1/8
    pre-scaled); kt/qt/vt stored bf16.
  V_aug via PE transpose: va8[128k, kb, {0:64,96:160}] = vt pair^T in
    fp8e4 (ones cols at 64/160 for the in-matmul denominator row); for
    kb<4 also vab in bf16 (qc=0 rows have few keys -> fp8 v-noise does
    not average out there).
  S^T[k, q] = K^T.T @ Q^T  (row-packed pair of K=64 bf16 matmuls)
  P^T = exp(S^T - 3)  (ACT; the -3 bias keeps exp <= e^4.8 ~ 121 < 240
    fp8e4 max; the uniform scale cancels in the softmax divide).
    qc=0: P^T in bf16, triangular gpsimd mask, bf16 AV (vab).
    qc>=1: P^T in fp8e4; full k-blocks as fp8 DoubleRow matmuls over
    (kb,kb+1) pairs (K=256/pass); diagonal blocks as single fp8 matmuls
    with the triangular gpsimd mask applied on the fp8 tile.
  out = po[0:64] * (1/denom) read directly from PSUM; denom row 64
    broadcast via DRAM round-trip DMA.
  Emission is software-pipelined: projection/V_aug units for chunk
  (p,qc+1) are interleaved between attention units of (p,qc) as PE
  filler, and the normalize multiply is deferred one qc so the denom DMA
  round-trip is hidden.
"""

import sys

sys.path.insert(0, "/opt/trn_rl_repo")

import numpy as np
import ml_dtypes

BF16NP = ml_dtypes.bfloat16
F8NP = ml_dtypes.float8_e4m3

B, N, D, H, HD = 4, 2048, 1024, 16, 64
NPAIR, DB, QC = 4, 8, 4  # head-pairs per core, 128-d-blocks, 512-q-chunks
SCALE = 1.0 / np.sqrt(HD)
EXPB = -3.0  # exp(s + EXPB): max causal score 7.8 -> exp <= 121 < 240

_PROG_CACHE = {}


def build_program(rep=1, dbg=False):
    from concourse import bacc
    import concourse.bass as bass
    import concourse.mybir as mybir
    from concourse.tile import TileContext

    F32, BF, F8 = mybir.dt.float32, mybir.dt.bfloat16, mybir.dt.float8e4
    DR = mybir.MatmulPerfMode.DoubleRow

    nc = bacc.Bacc("TRN2", target_bir_lowering=False)
    xt_d = nc.declare_dram_parameter("xt", [128, DB, N], BF, isOutput=False)
    wt_d = nc.declare_dram_parameter("wt", [NPAIR, 128, DB, 384], BF, isOutput=False)
    bias_d = nc.declare_dram_parameter("bias", [128, NPAIR, 3], F32, isOutput=False)
    mask_d = nc.declare_dram_parameter("mask", [128, 128], BF, isOutput=False)
    mask8_d = nc.declare_dram_parameter("mask8", [128, 128], F8, isOutput=False)
    id_d = nc.declare_dram_parameter("ident", [128, 128], BF, isOutput=False)
    out_d = nc.declare_dram_parameter("outt", [2 * NPAIR, 64, N], F32, isOutput=True)
    if dbg:
        dkt_d = nc.declare_dram_parameter("dkt", [128, N], BF, isOutput=True)
        dqt_d = nc.declare_dram_parameter("dqt", [128, N], BF, isOutput=True)
        dvt_d = nc.declare_dram_parameter("dvt", [128, N], BF, isOutput=True)
        dva_d = nc.declare_dram_parameter("dva", [128, 16, 192], F8, isOutput=True)
        dpt_d = nc.declare_dram_parameter("dpt", [128, 2, 1024], F8, isOutput=True)

    with TileContext(nc) as tc:
        with (
            tc.tile_pool(name="big", bufs=1) as big,
            tc.tile_pool(name="wp", bufs=2) as wp,
            tc.tile_pool(name="projp", bufs=2) as projp,
            tc.tile_pool(name="attp", bufs=3) as attp,
            tc.tile_pool(name="psA", bufs=2, space="PSUM") as psA,
            tc.tile_pool(name="dr", bufs=4, space="DRAM") as dr,
        ):
            xt = big.tile([128, DB, N], BF)
            for db in range(DB):
                nc.sync.dma_start(out=xt[:, db, :], in_=xt_d[:, db, :])
            mask = big.tile([128, 128], BF)
            nc.sync.dma_start(out=mask, in_=mask_d[:, :])
            mask8 = big.tile([128, 128], F8)
            nc.sync.dma_start(out=mask8, in_=mask8_d[:, :])
            ident = big.tile([128, 128], BF)
            nc.sync.dma_start(out=ident, in_=id_d[:, :])
            biasb = big.tile([128, NPAIR, 3], F32)
            nc.sync.dma_start(out=biasb, in_=bias_d[:, :, :])
            negc = big.tile([128, 1], F32)
            nc.vector.memset(negc, EXPB)

            # Persistent rings for the diagonal-pair fp8 P^T tiles: the dead
            # [0:q0] strips their DoubleRow AV streams through are zeroed ONCE
            # here; exp/mask writes never touch those strips, so the zeros
            # survive ring reuse (a tile pool would flag the cross-rotation
            # read as a race).
            ptp_ring = {"A": [], "B": []}
            for r in range(4):
                tA = attp.tile(
                    [128, 2, 1024], F8, tag=f"ptpA{r}", bufs=1, name=f"ptpA{r}"
                )
                for h2 in range(2):
                    nc.vector.memset(tA[:, 1, h2 * 512 : h2 * 512 + 128], 0)
                ptp_ring["A"].append(tA)
                tB = attp.tile(
                    [128, 2, 1024], F8, tag=f"ptpB{r}", bufs=1, name=f"ptpB{r}"
                )
                for h2 in range(2):
                    nc.vector.memset(tB[:, 0, h2 * 512 : h2 * 512 + 256], 0)
                    nc.vector.memset(tB[:, 1, h2 * 512 : h2 * 512 + 384], 0)
                ptp_ring["B"].append(tB)
            ptp_rc = {"A": 0, "B": 0}

            # ---- per-(rep, pair) state -------------------------------------
            class PairState:
                def __init__(self, p):
                    self.p = p
                    self.w = wp.tile([128, DB, 384], BF, tag="w")
                    nc.sync.dma_start(out=self.w, in_=wt_d[p])
                    self.kt = projp.tile([128, N], BF, tag="kt")
                    self.qt = projp.tile([128, N], BF, tag="qt")
                    self.vt = projp.tile([128, N], BF, tag="vt")
                    self.dests = [self.kt, self.qt, self.vt]
                    # head h2 V^T block at offset 96*h2 (ones col at +64);
                    # 96-stride keeps the DoubleRow weight AP 16B-aligned
                    self.va8 = attp.tile([128, 16, 192], F8, tag="va8", bufs=2)
                    nc.vector.memset(self.va8[:, :, 64:65], 1.0)
                    nc.vector.memset(self.va8[:, :, 160:161], 1.0)
                    # bf16 copy for qc=0 (kb<4 only), heads at 0/80
                    self.vab = attp.tile([128, 4, 160], BF, tag="vab", bufs=2)
                    nc.vector.memset(self.vab[:, :, 64:65], 1.0)
                    nc.vector.memset(self.vab[:, :, 144:145], 1.0)
                    self.outts = [
                        attp.tile([64, N], F32, tag=f"outt{h2}", bufs=2, name=f"outt{h2}")
                        for h2 in range(2)
                    ]

            def emit_proj_half(st, m, n4, half, pp):
                # half 0: db 0-3 (start), half 1: db 4-7 (stop) + bias add.
                # Split so filler insertions between attention units
                # stay under ~1us and never drain the S->exp pipeline.
                for db in range(4 * half, 4 * half + 4):
                    nc.tensor.matmul(
                        pp,
                        st.w[:, db, m * 128 : (m + 1) * 128],
                        xt[:, db, n4 * 512 : (n4 + 1) * 512],
                        start=(db == 0),
                        stop=(db == DB - 1),
                    )
                if half == 1:
                    nc.vector.tensor_scalar_add(
                        st.dests[m][:, n4 * 512 : (n4 + 1) * 512],
                        pp,
                        biasb[:, st.p, m : m + 1],
                    )

            def emit_proj_unit(st, m, n4):
                pp = psA.tile([128, 512], F32, tag="proj")
                emit_proj_half(st, m, n4, 0, pp)
                emit_proj_half(st, m, n4, 1, pp)

            def emit_vaug2(st, kb0):
                # va8[128k, kb, {0:64,96:160}] = vt[{h}*64:..., kb block]^T
                # Two kb transposes back-to-back (PE pipelines transpose-mode
                # better without interleaved deps), then the strided DVE
                # copies (bf16 psum -> fp8 sbuf; both head slots per copy).
                pvs = []
                for kb in (kb0, kb0 + 1):
                    pv = psA.tile([128, 128], BF, tag="proj", name="pv")
                    nc.tensor.transpose(
                        pv, st.vt[:, kb * 128 : (kb + 1) * 128], ident
                    )
                    pvs.append(pv)
                for kb, pv in zip((kb0, kb0 + 1), pvs):
                    src = pv[:, :].rearrange("p (h w) -> p h w", h=2)
                    dst8 = st.va8[:, kb, 0:192].rearrange(
                        "p (h w) -> p h w", h=2
                    )[:, :, 0:64]
                    nc.vector.tensor_copy(dst8, src)
                    if kb < 4:
                        dstb = st.vab[:, kb, 0:160].rearrange(
                            "p (h w) -> p h w", h=2
                        )[:, :, 0:64]
                        nc.vector.tensor_copy(dstb, src)

            def emit_s_unit(st, qc, kb, ptt, j):
                # S^T for one kb; exp -> ptt slot j ([:, j, :] for pair
                # tiles, whole tile for diag). fp8/bf16 set by ptt dtype.
                q0 = 128 * max(0, kb - 4 * qc)
                ps_s = psA.tile([128, 1024], F32, tag="s")
                for h2 in range(2):
                    nc.tensor.matmul(
                        ps_s[:, h2 * 512 + q0 : (h2 + 1) * 512],
                        st.kt[h2 * 64 : (h2 + 1) * 64, kb * 128 : (kb + 1) * 128],
                        st.qt[
                            h2 * 64 : (h2 + 1) * 64,
                            qc * 512 + q0 : (qc + 1) * 512,
                        ],
                        start=True,
                        stop=True,
                        tile_position=(h2 * 64, 0),
                    )
                dstv = ptt[:, j, :] if j is not None else ptt[:, :]
                if q0 == 0:
                    nc.scalar.activation(
                        dstv, ps_s[:, :],
                        mybir.ActivationFunctionType.Exp, bias=negc[:],
                    )
                else:
                    sv = ps_s[:, :].rearrange("p (b w) -> p b w", b=2)
                    tv = dstv.rearrange("p (b w) -> p b w", b=2)
                    nc.scalar.activation(
                        tv[:, :, q0:512], sv[:, :, q0:512],
                        mybir.ActivationFunctionType.Exp, bias=negc[:],
                    )
                if kb >= 4 * qc:  # diagonal block: triangular mask
                    mk = mask if qc == 0 else mask8
                    for h2 in range(2):
                        s = h2 * 512 + q0
                        nc.gpsimd.tensor_mul(
                            dstv[:, s : s + 128], dstv[:, s : s + 128], mk
                        )
                return q0

            def emit_av(unit):
                if unit[0] == "pair":
                    _, st, kb0, po, nkb, ptp = unit
                    for h2 in range(2):
                        nc.tensor.matmul(
                            po[h2][0:65, 0:512],
                            st.va8[:, kb0 : kb0 + 2, 96 * h2 : 96 * h2 + 65],
                            ptp[:, :, h2 * 512 : (h2 + 1) * 512],
                            start=(kb0 == 0),
                            stop=(kb0 == nkb - 2),
                            perf_mode=DR,
                        )
                else:  # qc == 0 only: bf16 AV against vab
                    _, st, qc, kb, po, nkb, pt, q0 = unit
                    for h2 in range(2):
                        nc.tensor.matmul(
                            po[h2][0:65, q0:512],
                            st.vab[:, kb, 80 * h2 : 80 * h2 + 65],
                            pt[:, h2 * 512 + q0 : (h2 + 1) * 512],
                            start=(kb == 0),
                            stop=(kb == nkb - 1),
                        )

            def emit_qc_tail(st, qc, po):
                # denom broadcast round-trip straight from PSUM row 64; po
                # stays live until emit_norm reads rows 0:64 one qc later.
                pend = []
                for h2 in range(2):
                    dnr = attp.tile([1, 512], F32, tag="dnr", bufs=6)
                    nc.vector.tensor_copy(dnr, po[h2][64:65, :])
                    recd = dr.tile([512], F32, tag="recd")
                    nc.sync.dma_start(
                        out=recd[:].rearrange("(a b) -> a b", a=1),
                        in_=dnr[:, :],
                    )
                    rb = attp.tile([64, 512], F32, tag="recb", bufs=6)
                    nc.sync.dma_start(out=rb, in_=recd[:].partition_broadcast(64))
                    pend.append((st, qc, h2, po, rb))
                return pend

            def emit_norm(pn):
                st, qc, h2, po, rb = pn
                nc.vector.reciprocal_approx_fast(out=rb, in_=rb)
                nc.vector.tensor_mul(
                    st.outts[h2][:, qc * 512 : (qc + 1) * 512],
                    po[h2][0:64, :],
                    rb,
                )
                if qc == QC - 1:
                    nc.sync.dma_start(out=out_d[2 * st.p + h2], in_=st.outts[h2])

            # ---- software-pipelined emission --------------------------------
            # flat sequence of (rep, pair); fillers for position i+1 emitted
            # interleaved inside the attention unit loops of position i.
            seq = [(r, p) for r in range(rep) for p in range(NPAIR)]
            states = {}

            def fillers_for(idx, qc_next):
                """Emission closures for proj/vaug of chunk following (idx,qc)."""
                if qc_next < QC:
                    key, nn = idx, qc_next
                else:
                    key, nn = idx + 1, 0
                    if key >= len(seq):
                        return []
                if key not in states:
                    states[key] = PairState(seq[key][1])
                st = states[key]
                units = []
                for m in range(3):
                    holder = []

                    def h0(m=m, st=st, nn=nn, holder=holder):
                        holder.append(psA.tile([128, 512], F32, tag="proj", name="pp"))
                        emit_proj_half(st, m, nn, 0, holder[0])

                    def h1(m=m, st=st, nn=nn, holder=holder):
                        emit_proj_half(st, m, nn, 1, holder[0])

                    units += [h0, h1]
                units.append(lambda st=st, nn=nn: emit_vaug2(st, 4 * nn))
                units.append(lambda st=st, nn=nn: emit_vaug2(st, 4 * nn + 2))
                return units

            # PE warmup: ~5us of junk matmuls while the xt DMA streams in, so
            # HAM un-throttles (1.2->2.4GHz) before the first real projection.
            wu = psA.tile([128, 128], F32, tag="proj", name="warm")
            for _ in range(48):
                nc.tensor.matmul(wu, ident, ident, start=True, stop=True)

            # prologue: first chunk's proj + vaug
            states[0] = PairState(seq[0][1])
            for m in range(3):
                emit_proj_unit(states[0], m, 0)
            emit_vaug2(states[0], 0)
            emit_vaug2(states[0], 2)

            AVLAG = 3
            pending_norms = []
            pend_av = []
            pend_tail = None
            for idx in range(len(seq)):
                st = states[idx]
                for qc in range(QC):
                    nkb = 4 * qc + 4
                    if qc == 0:
                        kinds = [("diag", 0, kb) for kb in range(4)]
                    else:
                        kinds = [("pair", "f", kb0) for kb0 in range(0, 4 * qc, 2)]
                        kinds += [("pair", "A", 4 * qc), ("pair", "B", 4 * qc + 2)]
                    filler = fillers_for(idx, qc + 1)
                    # spread filler units fractionally across the whole unit
                    # loop (integer step bunches them early and leaves the
                    # tail units naked -> ACT-limited stalls soak into AV durs)
                    points = {}
                    if filler:
                        nf = len(filler)
                        nu = len(kinds)
                        for i, f in enumerate(filler):
                            up = min(nu - 1, 1 + (i * nu) // nf)
                            points.setdefault(up, []).append(f)
                    po = [
                        psA.tile([128, 512], F32, tag="o", name=f"po{h2}")
                        for h2 in range(2)
                    ]
                    # AV lags S/exp by AVLAG units — ACROSS qc boundaries — so
                    # an exp-waiting AV always has the next S groups ahead of
                    # it in the PE FIFO. The qc tail (denom DMAs) is deferred
                    # to the next qc's slot AVLAG-1, right after the AV flush
                    # of this qc's last unit, so it still depends on (and is
                    # emitted after) the last AV of this qc.
                    for ui, (kind, cls, kb) in enumerate(kinds):
                        if kind == "pair":
                            if cls == "f":
                                ptp = attp.tile(
                                    [128, 2, 1024], F8, tag="ptp", bufs=6
                                )
                            else:
                                ptp = ptp_ring[cls][ptp_rc[cls] % 4]
                                ptp_rc[cls] += 1
                            emit_s_unit(st, qc, kb, ptp, 0)
                            emit_s_unit(st, qc, kb + 1, ptp, 1)
                            if dbg and st.p == 0 and qc == 1 and kb == 0:
                                nc.sync.dma_start(out=dpt_d[:], in_=ptp)
                            cur = ("pair", st, kb, po, nkb, ptp)
                        else:
                            pt = attp.tile([128, 1024], BF, tag="ptb", bufs=6)
                            q0 = emit_s_unit(st, qc, kb, pt, None)
                            cur = ("diag", st, qc, kb, po, nkb, pt, q0)
                        if len(pend_av) >= AVLAG:
                            emit_av(pend_av.pop(0))
                        pend_av.append(cur)
                        if ui == AVLAG - 1 and pend_tail is not None:
                            new_pend = emit_qc_tail(*pend_tail)
                            pend_tail = None
                            for pn in pending_norms:
                                emit_norm(pn)
                            pending_norms = new_pend
                        for f in points.get(ui, ()):
                            f()
                    pend_tail = (st, qc, po)
                if dbg and idx == 0:
                    nc.sync.dma_start(out=dkt_d[:, :], in_=st.kt)
                    nc.sync.dma_start(out=dqt_d[:, :], in_=st.qt)
                    nc.sync.dma_start(out=dvt_d[:, :], in_=st.vt)
                    nc.sync.dma_start(out=dva_d[:], in_=st.va8)
                del states[idx]
            for u in pend_av:
                emit_av(u)
            if pend_tail is not None:
                for pn in pending_norms:
                    emit_norm(pn)
                pending_norms = emit_qc_tail(*pend_tail)
            for pn in pending_norms:
                emit_norm(pn)

    nc.compile()
    return nc


def get_program(rep=1):
    if rep not in _PROG_CACHE:
        _PROG_CACHE[rep] = build_program(rep)
    return _PROG_CACHE[rep]


def prep_inputs(x, W, b):
    x = np.asarray(x, dtype=np.float32)
    W = np.asarray(W, dtype=np.float32)
    b = np.asarray(b, dtype=np.float32)
    mask = (np.arange(128)[:, None] <= np.arange(128)[None, :]).astype(BF16NP)
    mask8 = mask.astype(F8NP)
    ident = np.eye(128).astype(BF16NP)

    in_maps = []
    for c in range(8):
        bc, hg = divmod(c, 2)
        xt = np.ascontiguousarray(
            x[bc].T.reshape(DB, 128, N).transpose(1, 0, 2)
        ).astype(BF16NP)  # [128(dlow), DB, N]
        wt = np.empty((NPAIR, 128, DB, 384), np.float32)
        bias = np.empty((128, NPAIR, 3), np.float32)
        for p in range(NPAIR):
            g1, g2 = hg * 8 + 2 * p, hg * 8 + 2 * p + 1
            Wp = np.empty((D, 384), np.float32)
            Wp[:, 0:64] = W[g1, :, 0:64]
            Wp[:, 64:128] = W[g2, :, 0:64]
            Wp[:, 128:192] = W[g1, :, 64:128] * SCALE
            Wp[:, 192:256] = W[g2, :, 64:128] * SCALE
            Wp[:, 256:320] = W[g1, :, 128:192]
            Wp[:, 320:384] = W[g2, :, 128:192]
            wt[p] = Wp.reshape(DB, 128, 384).transpose(1, 0, 2)
            bias[0:64, p, 0] = b[g1, 0:64]
            bias[64:128, p, 0] = b[g2, 0:64]
            bias[0:64, p, 1] = b[g1, 64:128] * SCALE
            bias[64:128, p, 1] = b[g2, 64:128] * SCALE
            bias[0:64, p, 2] = b[g1, 128:192]
            bias[64:128, p, 2] = b[g2, 128:192]
        in_maps.append(
            {"xt": xt, "wt": wt.astype(BF16NP), "bias": bias, "mask": mask,
             "mask8": mask8, "ident": ident}
        )
    return in_maps


def run(nc, in_maps):
    from concourse.bass_utils import run_bass_kernel_spmd

    return run_bass_kernel_spmd(nc, in_maps, list(range(8)))


class Runner:
    """Persistent PJRT executable for an nc program: loads the NEFF once and
    reuses it across calls (run_bass_via_pjrt reloads per call)."""

    def __init__(self, nc, n_cores=8):
        import jax
        import numpy as np
        from jax.sharding import Mesh, PartitionSpec
        from jax.experimental.shard_map import shard_map
        import concourse.mybir as mybir
        from concourse import bass2jax

        bass2jax.install_neuronx_cc_hook()
        self.n_cores = n_cores
        partition_name = (
            nc.partition_id_tensor.name if nc.partition_id_tensor else None
        )
        in_names, out_names, out_avals, zero_outs = [], [], [], []
        for alloc in nc.m.functions[0].allocations:
            if not isinstance(alloc, mybir.MemoryLocationSet):
                continue
            name = alloc.memorylocations[0].name
            if alloc.kind == "ExternalInput":
                if name != partition_name:
                    in_names.append(name)
            elif alloc.kind == "ExternalOutput":
                shape = tuple(alloc.tensor_shape)
                dtype = mybir.dt.np(alloc.dtype)
                out_names.append(name)
                out_avals.append(jax.core.ShapedArray(shape, dtype))
                zero_outs.append(np.zeros(shape, dtype))
        n_params = len(in_names)
        all_in_names = list(in_names) + list(out_names)
        if partition_name is not None:
            all_in_names.append(partition_name)

        def _body(*args):
            operands = list(args)
            if partition_name is not None:
                operands.append(bass2jax.partition_id_tensor())
            outs = bass2jax._bass_exec_p.bind(
                *operands,
                out_avals=tuple(out_avals),
                in_names=tuple(all_in_names),
                out_names=tuple(out_names),
                lowering_input_output_aliases=(),
                sim_require_finite=True,
                sim_require_nnan=True,
                nc=nc,
            )
            return tuple(outs)

        devices = jax.devices()[:n_cores]
        mesh = Mesh(np.asarray(devices), ("core",))
        in_specs = (PartitionSpec("core"),) * (n_params + len(out_names))
        out_specs = (PartitionSpec("core"),) * len(out_names)
        self._fn = jax.jit(
            shard_map(
                _body,
                mesh=mesh,
                in_specs=in_specs,
                out_specs=out_specs,
                check_rep=False,
            ),
            keep_unused=True,
        )
        self.in_names, self.out_names = in_names, out_names
        self.out_avals, self.zero_outs = out_avals, zero_outs
        self.n_params = n_params
        self._jax = jax

    def put_inputs(self, in_maps):
        import numpy as np

        concat_in = [
            np.concatenate([np.asarray(m[n]) for m in in_maps], axis=0)
            for n in self.in_names
        ]
        concat_zeros = [
            np.zeros((self.n_cores * z.shape[0], *z.shape[1:]), z.dtype)
            for z in self.zero_outs
        ]
        return [self._jax.device_put(a) for a in concat_in + concat_zeros]

    def execute(self, dev_args):
        outs = self._fn(*dev_args)
        self._jax.block_until_ready(outs)
        return outs

    def run(self, in_maps):
        import numpy as np

        outs = self.execute(self.put_inputs(in_maps))
        return [
            {
                n: np.asarray(outs[i]).reshape(
                    self.n_cores, *self.out_avals[i].shape
                )[c]
                for i, n in enumerate(self.out_names)
            }
            for c in range(self.n_cores)
        ]


def assemble(results):
    out = np.empty((B, N, D), np.float32)
    for c in range(8):
        bc, hg = divmod(c, 2)
        outt = results[c]["outt"]  # [8, 64, N]
        for hh in range(8):
            out[bc, :, hg * 512 + hh * 64 : hg * 512 + (hh + 1) * 64] = outt[hh].T
    return out


def kernel(x, W, b):
    nc = get_program(rep=1)
    in_maps = prep_inputs(x, W, b)
    out = assemble(run(nc, in_maps).results)
    # transient device-state glitches (seen ~once per ~20 session runs after
    # a wedged-device recovery) produce catastrophic garbage; retry once
    if not np.isfinite(out).all() or np.abs(out).max() > 100.0:
        out = assemble(run(nc, in_maps).results)
    return out
